# revision 1
# baseline (speedup 1.0000x reference)
"""Block-circulant matmul kernel for Trainium2 (8 NeuronCores, data-parallel).

Computes out = (x * D) @ M + bias where M is the 4096x4096 block-circulant
matrix built from W[32, 32, 128] (block (i,j) is C_ij[s,t] = W[i,j,(s-t)%128]).

Sharding: batch (4096) split 8 ways -> 512 rows per core; weights replicated.

Device algorithm (per core): 3-stage frequency-domain factorization.
  A: DFT-as-matmul (shared Csig weight, 32 matmuls over j blocks)
  T1: DVE 32x32 stream-transpose (quadrant-local c<->j swap)
  B: per-frequency-slot block-diagonal matmul (32 slots)
  T2: DVE stream-transpose (i<->c swap)
  C: iDFT-as-matmul + bias
The sigma frequency packing puts the 4 real components of a frequency
pair-slot c at spectrum positions {c, 32+c, 64+c, 96+c} so the quadrant-local
DVE transpose lands rows exactly where the next stage's matmul needs them.

Performance notes baked in here:
 - Whole dataflow in bf16 (PSUM accumulation stays fp32): 1 cyc/row PE
   matmuls, half DMA/SBUF traffic, and fast packed DVE transposes.
 - Transposed tensors keep the 32-wide transposed axis innermost and
   contiguous in BOTH operands: packed stream transpose measured ~2x faster
   than the strided form (and bf16 packed is faster still).
 - Emission order pipelines PE/ACT work under the DVE transpose chain.
 - Output is written bf16 into [BS, KO, BC]-layout DRAM in 4 grouped DMAs
   (8KB contiguous runs) and upcast host-side.
"""

import os
import numpy as np

import concourse.bass as bass
import concourse.mybir as mybir
from concourse import bacc
from concourse.tile import TileContext
from concourse.bass_utils import run_bass_kernel_spmd
import concourse.bass_utils as _bu

# Walrus flag rewrites for this kernel's own compiles:
#  - LDWOPT: let walrus overlap LDWEIGHTS with in-flight matmuls.
#  - SKIP_BIRVER: drop the birverifier pass (needed only for f32r fallback
#    modes where the DVE transpose cannot carry the f32r dtype tag walrus
#    insists on; harmless otherwise). Correctness is checked end-to-end.
LDWOPT = os.environ.get("BC_LDWOPT", "0") == "1"
SKIP_BIRVER = os.environ.get("BC_SKIP_BIRVER", "1") == "1"
if not getattr(_bu, "_bc_ldwopt_patched", False):
    _bu._bc_ldwopt_patched = True
    _orig_bvo = _bu.bir_verify_and_optimise

    def _bvo_ldwopt(*a, **k):
        orig_rc = _bu.run_command

        def rc(argv, **kw):
            def rw(s):
                if LDWOPT:
                    s = s.replace("--enable-ldw-opt=false",
                                  "--enable-ldw-opt=true")
                if SKIP_BIRVER and s.startswith("birverifier,"):
                    s = s[len("birverifier,"):]
                return s

            return orig_rc([rw(s) for s in argv], **kw)

        _bu.run_command = rc
        try:
            return _orig_bvo(*a, **k)
        finally:
            _bu.run_command = orig_rc

    _bu.bir_verify_and_optimise = _bvo_ldwopt

# Problem constants (hardcoded per harness contract).
BATCH = 4096
D_IN = 4096
D_OUT = 4096
BS = 128          # circulant block size
KI = 32           # input blocks
KO = 32           # output blocks
NCORES = 8
BC = BATCH // NCORES      # 512 batch rows per core
NSPLIT = 2                # batch halves per core (pipeline + PSUM sizing)
BH = BC // NSPLIT

OGRP = 8                  # output blocks per grouped store DMA

_NC_CACHE = {}
_PACK_CACHE = {}


# ---------------------------------------------------------------- sigma pack
def _sigma_components():
    """slot c, quadrant Q -> ("re"|"im", f). Pairs (2c+1, 2c+2) for c<31,
    slot 31 holds (63 complex, 0 real, 64 real)."""
    comp = {}
    for c in range(32):
        fa = 2 * c + 1 if c < 31 else 63
        comp[(0, c)] = ("re", fa)
        comp[(1, c)] = ("im", fa)
        if c < 31:
            comp[(2, c)] = ("re", 2 * c + 2)
            comp[(3, c)] = ("im", 2 * c + 2)
        else:
            comp[(2, c)] = ("re", 0)
            comp[(3, c)] = ("re", 64)
    return comp


def _pack_const():
    """Input-independent factor matrices Csig [s, m] and Esig [m, t]."""
    if "const" in _PACK_CACHE:
        return _PACK_CACHE["const"]
    comp = _sigma_components()
    s = np.arange(BS)
    Csig = np.zeros((BS, 128), dtype=np.float64)
    Esig = np.zeros((128, BS), dtype=np.float64)
    for (Q, c), (typ, f) in comp.items():
        m = 32 * Q + c
        ang = 2 * np.pi * f * s / BS
        a = (1.0 if f in (0, 64) else 2.0) / BS
        if typ == "re":
            Csig[:, m] = np.cos(ang)
            Esig[m, :] = a * np.cos(ang)
        else:
            Csig[:, m] = -np.sin(ang)
            Esig[m, :] = -a * np.sin(ang)
    out = (Csig.astype(np.float32), np.ascontiguousarray(Esig.astype(np.float32)))
    _PACK_CACHE["const"] = out
    return out


def _pack_wb(W):
    """Frequency-domain block-diagonal weights WBt [row=(Qr,j), slot, col=(Qc,i)]."""
    comp = _sigma_components()
    Wf = np.fft.fft(W.astype(np.float64), axis=-1)
    Wfr, Wfi = Wf.real, Wf.imag
    WB = np.zeros((32, 128, 128), dtype=np.float64)
    for c in range(32):
        for (qre, qim) in ((0, 1), (2, 3)):
            typ_im = comp[(qim, c)][0]
            f = comp[(qre, c)][1]
            if typ_im == "im":
                wr = Wfr[:, :, f].T  # [j, i]
                wi = Wfi[:, :, f].T
                WB[c, qre*32:(qre+1)*32, qre*32:(qre+1)*32] = wr
                WB[c, qim*32:(qim+1)*32, qre*32:(qre+1)*32] = wi
                WB[c, qre*32:(qre+1)*32, qim*32:(qim+1)*32] = -wi
                WB[c, qim*32:(qim+1)*32, qim*32:(qim+1)*32] = wr
            else:
                f2 = comp[(qim, c)][1]
                WB[c, qre*32:(qre+1)*32, qre*32:(qre+1)*32] = Wfr[:, :, f].T
                WB[c, qim*32:(qim+1)*32, qim*32:(qim+1)*32] = Wfr[:, :, f2].T
    return np.ascontiguousarray(
        WB.transpose(1, 0, 2).astype(np.float32)  # [row, slot, col]
    )


# ---------------------------------------------------------------- fft build
def _build_fft():
    if "fft" in _NC_CACHE:
        return _NC_CACHE["fft"]
    f32 = mybir.dt.float32
    bf16 = mybir.dt.bfloat16
    nsplit = NSPLIT
    bh = BH

    nc = bacc.Bacc(None, target_bir_lowering=False, debug=False)

    # D_bernoulli is folded into x host-side, so stage A shares one Csig
    # weight across all j-matmuls.
    xT = nc.dram_tensor("xT", [BS, KI, BC], bf16, kind="ExternalInput")
    Csig_d = nc.dram_tensor("Csig", [BS, 128], bf16, kind="ExternalInput")
    WBt_d = nc.dram_tensor("WBt", [128, 32, 128], bf16, kind="ExternalInput")
    Esig_d = nc.dram_tensor("Esig", [128, BS], bf16, kind="ExternalInput")
    bT_d = nc.dram_tensor("bT", [BS, KO], f32, kind="ExternalInput")
    # [s, i, b] layout: grouped stores get 8KB contiguous runs per partition.
    outT = nc.dram_tensor("outT", [BS, KO, BC], bf16, kind="ExternalOutput")
    if LDWOPT:
        nc.dram_tensor("ldwopt_tag", [1, 1], f32, kind="ExternalInput")

    def do_copy(out, in_, eng):
        if eng == "v":
            nc.vector.tensor_copy(out=out, in_=in_)
        else:
            nc.scalar.activation(
                out=out, in_=in_, func=mybir.ActivationFunctionType.Copy
            )

    with TileContext(nc) as tc:
        with tc.tile_pool(name="consts", bufs=1) as cpool, \
             tc.tile_pool(name="stage", bufs=6) as spool, \
             tc.tile_pool(name="big1", bufs=2) as big1, \
             tc.tile_pool(name="big2", bufs=2) as big2, \
             tc.tile_pool(name="og", bufs=KO // OGRP) as ogpool, \
             tc.tile_pool(name="psAll", bufs=8, space="PSUM") as psAll:

            csig = cpool.tile([BS, 128], bf16)
            esig = cpool.tile([128, BS], bf16)
            bt_t = cpool.tile([BS, KO], f32)
            wb = cpool.tile([128, 32, 128], bf16)
            nc.sync.dma_start(out=csig, in_=Csig_d[:, :])
            nc.sync.dma_start(out=esig, in_=Esig_d[:, :])
            nc.sync.dma_start(out=bt_t, in_=bT_d[:, :])
            nc.sync.dma_start(out=wb, in_=WBt_d[:, :, :])

            # Output group tiles: oc[g][:, i-8g, :] collects block i (full BC
            # width); one DMA per group once both batch halves are in.
            oc = [ogpool.tile([BS, OGRP, BC], bf16, tag="og", name=f"oc{g}")
                  for g in range(KO // OGRP)]

            # ---- stage A: spectrum XF[m, b, j] (j innermost+contiguous so
            # T1 runs in the packed fast mode).
            xf = [big1.tile([128, bh, KI], bf16, tag="big1", name=f"xf{h}")
                  for h in range(nsplit)]
            # j-paired input DMAs (2KB runs); full-width (N=512) matmuls;
            # psum copied out per (j, h) half, alternating DVE/ACT.
            for jp in range(KI // 2):
                st = spool.tile([BS, 2, BC], bf16, tag="stage")
                nc.sync.dma_start(out=st, in_=xT[:, 2 * jp : 2 * jp + 2, :])
                for k in range(2):
                    j = 2 * jp + k
                    ps = psAll.tile([128, BC], f32, tag="ps", name=f"psa{j}")
                    nc.tensor.matmul(
                        ps, csig[:, :], st[:, k, :], start=True, stop=True
                    )
                    for h in range(nsplit):
                        do_copy(xf[h][:, :, j],
                                ps[:, h * bh : (h + 1) * bh],
                                "v" if (j + h) % 2 == 0 else "a")

            # ---- T1 (both halves back-to-back on DVE): Z[(Q,j), b, c] =
            # XF[(Q,c), b, j], quadrant-local 32x32 blocks, packed bf16.
            z = [big2.tile([128, bh, 32], bf16, tag="big2", name=f"z{h}")
                 for h in range(nsplit)]
            for h in range(nsplit):
                nc.vector.transpose(out=z[h], in_=xf[h])

            yz = [None] * nsplit
            yw = [None] * nsplit

            def stage_b(h):
                # per-slot block-diagonal frequency matmul
                yz[h] = big1.tile([128, bh, 32], bf16, tag="big1", name=f"yz{h}")
                for c in range(32):
                    ps = psAll.tile([128, bh], f32, tag="ps", name=f"psb{c}_{h}")
                    nc.tensor.matmul(
                        ps, wb[:, c, :], z[h][:, :, c], start=True, stop=True
                    )
                    do_copy(yz[h][:, :, c], ps, "v" if c % 4 == 0 else "a")

            def t2(h):
                # YW[(Q,c), b, i] = YZ[(Q,i), b, c]
                yw[h] = big2.tile([128, bh, 32], bf16, tag="big2", name=f"yw{h}")
                nc.vector.transpose(out=yw[h], in_=yz[h])

            def stage_c(h):
                # iDFT + bias; esig shared, two output blocks per matmul.
                for i in range(0, KO, 2):
                    ps = psAll.tile([128, bh, 2], f32, tag="ps", name=f"psc{i}_{h}")
                    nc.tensor.matmul(
                        ps, esig[:, :], yw[h][:, :, i : i + 2],
                        start=True, stop=True,
                    )
                    for d in range(2):
                        g, r = (i + d) // OGRP, (i + d) % OGRP
                        dst = oc[g][:, r, h * bh : (h + 1) * bh]
                        if (i // 2 + d) % 2 == 0:
                            nc.scalar.activation(
                                out=dst, in_=ps[:, :, d],
                                func=mybir.ActivationFunctionType.Identity,
                                bias=bt_t[:, i + d : i + d + 1],
                            )
                        else:
                            nc.vector.tensor_scalar_add(
                                out=dst, in0=ps[:, :, d],
                                scalar1=bt_t[:, i + d : i + d + 1],
                            )

            # Emission order keeps PE/ACT busy under the DVE transpose chain:
            # B(0) runs during T1(1), B(1) during T2(0), C(0) during T2(1).
            stage_b(0)
            t2(0)
            stage_b(1)
            stage_c(0)
            t2(1)
            stage_c(1)

            for g in range(KO // OGRP):
                nc.sync.dma_start(
                    out=outT[:, g * OGRP : (g + 1) * OGRP, :], in_=oc[g]
                )

    nc.compile()
    _NC_CACHE["fft"] = nc
    return nc


def _prep_fft(x, W, D, bias):
    import ml_dtypes
    bf16 = ml_dtypes.bfloat16
    Csig, Esig = _pack_const()
    WBt = _pack_wb(W)
    xd = (x * D[None, :]).astype(bf16)  # fold Bernoulli diagonal host-side
    bT = np.ascontiguousarray(bias.reshape(KO, BS).T)
    Csig16 = Csig.astype(bf16)
    Esig16 = Esig.astype(bf16)
    WBt16 = WBt.astype(bf16)
    in_maps = []
    for c in range(NCORES):
        xs = xd[c * BC : (c + 1) * BC, :]
        xTc = np.ascontiguousarray(xs.reshape(BC, KI, BS).transpose(2, 1, 0))
        im = {"xT": xTc, "Csig": Csig16, "WBt": WBt16, "Esig": Esig16, "bT": bT}
        if LDWOPT:
            im["ldwopt_tag"] = np.zeros((1, 1), dtype=np.float32)
        in_maps.append(im)
    return in_maps


# ------------------------------------------------------------------- driver
def _run(inputs, trace=False):
    x = np.asarray(inputs["x"], dtype=np.float32)
    W = np.asarray(inputs["W"], dtype=np.float32)
    D = np.asarray(inputs["D_bernoulli"], dtype=np.float32)
    bias = np.asarray(inputs["bias"], dtype=np.float32)

    nc = _build_fft()
    in_maps = _prep_fft(x, W, D, bias)

    res = run_bass_kernel_spmd(nc, in_maps, list(range(NCORES)), trace=trace)
    out = np.empty((BATCH, D_OUT), dtype=np.float32)
    for c in range(NCORES):
        oT = np.asarray(res.results[c]["outT"]).astype(np.float32)  # [s, i, b]
        out[c * BC : (c + 1) * BC, :] = (
            oT.transpose(2, 1, 0).reshape(BC, D_OUT)
        )
    return out, res


def kernel(**inputs) -> np.ndarray:
    out, _ = _run(inputs, trace=False)
    return out



# revision 10
# speedup vs baseline: 1.5128x; 1.5128x over previous
"""Block-circulant matmul kernel for Trainium2 (8 NeuronCores, data-parallel).

Computes out = (x * D) @ M + bias where M is the 4096x4096 block-circulant
matrix built from W[32, 32, 128] (block (i,j) is C_ij[s,t] = W[i,j,(s-t)%128]).

Sharding: batch (4096) split 8 ways -> 512 rows per core; weights replicated.

Device algorithm (per core): 3-stage frequency-domain factorization.
  A: DFT-as-matmul with (b16, j32)-mixed moving free dims, so PSUM holds
     XF[m, b, j] with j innermost; Scalar/GpSimd drain it to SBUF with
     CONTIGUOUS cast-copies (the HW stream transpose cannot cast:
     s4d4_tr_same_src_dst_type, so a bf16 SBUF staging tile is needed).
  T1: packed bf16 DVE stream-transpose xf -> z[(Q,j), b, c] in quarters.
  B: per-frequency-slot block-diagonal matmul; 4 slots share one 2-bank
     PSUM tile so the drain copy has 4-wide contiguous runs; drains on
     Scalar/GpSimd (DVE stays free for transposes).
  T2: packed DVE stream-transpose yz -> yw in quarter-slabs.
  C: iDFT-as-matmul; pure-cast drains (bias is applied host-side, it is a
     linear post-op) on Scalar/GpSimd; quarter output tiles DMA out with
     8KB/partition contiguous runs.

The sigma frequency packing puts the 4 real components of a frequency
pair-slot c at spectrum positions {c, 32+c, 64+c, 96+c} so the quadrant-local
DVE transpose lands rows exactly where the next stage's matmul needs them.

Perf notes:
 - Whole dataflow bf16 (PSUM accumulation fp32).
 - The old kernel spent ~200us of Scalar/DVE time on strided-destination
   PSUM->SBUF copies; T1-from-PSUM plus c-quad PSUM grouping removes them.
 - DVE is the bottleneck engine (~40us of transposes); everything else is
   kept off DVE and ordered to hide under it.
"""

import os
import numpy as np

import concourse.bass as bass
import concourse.mybir as mybir
from concourse import bacc
from concourse.tile import TileContext
from concourse.bass_utils import run_bass_kernel_spmd
import concourse.bass_utils as _bu

# Walrus flag rewrites for this kernel's own compiles:
#  - LDWOPT: let walrus overlap LDWEIGHTS with in-flight matmuls.
#  - SKIP_BIRVER: drop the birverifier pass (the dtype-casting DVE stream
#    transpose trips its tag checks; correctness is checked end-to-end).
LDWOPT = os.environ.get("BC_LDWOPT", "0") == "1"
SKIP_BIRVER = os.environ.get("BC_SKIP_BIRVER", "1") == "1"
if not getattr(_bu, "_bc_ldwopt_patched", False):
    _bu._bc_ldwopt_patched = True
    _orig_bvo = _bu.bir_verify_and_optimise

    def _bvo_ldwopt(*a, **k):
        orig_rc = _bu.run_command

        def rc(argv, **kw):
            def rw(s):
                if LDWOPT:
                    s = s.replace("--enable-ldw-opt=false",
                                  "--enable-ldw-opt=true")
                if SKIP_BIRVER and s.startswith("birverifier,"):
                    s = s[len("birverifier,"):]
                return s

            return orig_rc([rw(s) for s in argv], **kw)

        _bu.run_command = rc
        try:
            return _orig_bvo(*a, **k)
        finally:
            _bu.run_command = orig_rc

    _bu.bir_verify_and_optimise = _bvo_ldwopt

# Problem constants (hardcoded per harness contract).
BATCH = 4096
D_IN = 4096
D_OUT = 4096
BS = 128          # circulant block size
KI = 32           # input blocks
KO = 32           # output blocks
NCORES = 8
BC = BATCH // NCORES      # 512 batch rows per core

NQ = 4                    # output quarters (b-blocks of 128)
BQ = BC // NQ             # 128
NH = 2                    # stage-B halves (b-blocks of 256)
BH = BC // NH             # 256
AB = 16                   # stage-A b-block per matmul (AB*KI = 512 free)
ABT = 32                  # stage-A b-block per T1 transpose (2 matmuls)
CQ = 4                    # stage-B slots per PSUM tile

_NC_CACHE = {}
_PACK_CACHE = {}


# ---------------------------------------------------------------- sigma pack
def _sigma_components():
    """slot c, quadrant Q -> ("re"|"im", f). Pairs (2c+1, 2c+2) for c<31,
    slot 31 holds (63 complex, 0 real, 64 real)."""
    comp = {}
    for c in range(32):
        fa = 2 * c + 1 if c < 31 else 63
        comp[(0, c)] = ("re", fa)
        comp[(1, c)] = ("im", fa)
        if c < 31:
            comp[(2, c)] = ("re", 2 * c + 2)
            comp[(3, c)] = ("im", 2 * c + 2)
        else:
            comp[(2, c)] = ("re", 0)
            comp[(3, c)] = ("re", 64)
    return comp


def _pack_const():
    """Input-independent factor matrices Csig [s, m] and Esig [m, t]."""
    if "const" in _PACK_CACHE:
        return _PACK_CACHE["const"]
    comp = _sigma_components()
    s = np.arange(BS)
    Csig = np.zeros((BS, 128), dtype=np.float64)
    Esig = np.zeros((128, BS), dtype=np.float64)
    for (Q, c), (typ, f) in comp.items():
        m = 32 * Q + c
        ang = 2 * np.pi * f * s / BS
        a = (1.0 if f in (0, 64) else 2.0) / BS
        if typ == "re":
            Csig[:, m] = np.cos(ang)
            Esig[m, :] = a * np.cos(ang)
        else:
            Csig[:, m] = -np.sin(ang)
            Esig[m, :] = -a * np.sin(ang)
    out = (Csig.astype(np.float32), np.ascontiguousarray(Esig.astype(np.float32)))
    _PACK_CACHE["const"] = out
    return out


def _pack_wb(W):
    """Frequency-domain block-diagonal weights WBt [row=(Qr,j), slot, col=(Qc,i)]."""
    comp = _sigma_components()
    Wf = np.fft.fft(W.astype(np.float64), axis=-1)
    Wfr, Wfi = Wf.real, Wf.imag
    WB = np.zeros((32, 128, 128), dtype=np.float64)
    for c in range(32):
        for (qre, qim) in ((0, 1), (2, 3)):
            typ_im = comp[(qim, c)][0]
            f = comp[(qre, c)][1]
            if typ_im == "im":
                wr = Wfr[:, :, f].T  # [j, i]
                wi = Wfi[:, :, f].T
                WB[c, qre*32:(qre+1)*32, qre*32:(qre+1)*32] = wr
                WB[c, qim*32:(qim+1)*32, qre*32:(qre+1)*32] = wi
                WB[c, qre*32:(qre+1)*32, qim*32:(qim+1)*32] = -wi
                WB[c, qim*32:(qim+1)*32, qim*32:(qim+1)*32] = wr
            else:
                f2 = comp[(qim, c)][1]
                WB[c, qre*32:(qre+1)*32, qre*32:(qre+1)*32] = Wfr[:, :, f].T
                WB[c, qim*32:(qim+1)*32, qim*32:(qim+1)*32] = Wfr[:, :, f2].T
    return np.ascontiguousarray(
        WB.transpose(1, 0, 2).astype(np.float32)  # [row, slot, col]
    )


# ---------------------------------------------------------------- build
def _build_fft():
    if "fft" in _NC_CACHE:
        return _NC_CACHE["fft"]
    f32 = mybir.dt.float32
    bf16 = mybir.dt.bfloat16

    nc = bacc.Bacc(None, target_bir_lowering=False, debug=False)

    # D_bernoulli is folded into x host-side; bias is applied host-side.
    xT = nc.dram_tensor("xT", [BS, BC, KI], bf16, kind="ExternalInput")
    Csig_d = nc.dram_tensor("Csig", [BS, 128], bf16, kind="ExternalInput")
    WBt_d = nc.dram_tensor("WBt", [128, 32, 128], bf16, kind="ExternalInput")
    Esig_d = nc.dram_tensor("Esig", [128, BS], bf16, kind="ExternalInput")
    # [s, q, i, bq]: each quarter DMA writes 8KB contiguous per partition.
    outT = nc.dram_tensor("outT", [BS, NQ, KO, BQ], bf16, kind="ExternalOutput")
    if LDWOPT:
        nc.dram_tensor("ldwopt_tag", [1, 1], f32, kind="ExternalInput")

    def drain(out, in_, eng):
        # GpSimd has no PSUM access on TRN2, so PSUM drains are split
        # between Scalar ("s") and DVE ("v") only.
        if eng == "s":
            nc.scalar.activation(
                out=out, in_=in_, func=mybir.ActivationFunctionType.Copy
            )
        else:
            nc.vector.tensor_copy(out=out, in_=in_)

    with TileContext(nc) as tc:
        with tc.tile_pool(name="consts", bufs=1) as cpool, \
             tc.tile_pool(name="xin", bufs=1) as xpool, \
             tc.tile_pool(name="xfb", bufs=1) as xfpool, \
             tc.tile_pool(name="zb", bufs=1) as zpool, \
             tc.tile_pool(name="yzb", bufs=1) as yzpool, \
             tc.tile_pool(name="ywb", bufs=1) as ywpool, \
             tc.tile_pool(name="oq", bufs=2) as oqpool, \
             tc.tile_pool(name="psA", bufs=2, space="PSUM") as psApool, \
             tc.tile_pool(name="psB", bufs=1, space="PSUM") as psBpool, \
             tc.tile_pool(name="psC", bufs=2, space="PSUM") as psCpool:

            csig = cpool.tile([BS, 128], bf16)
            esig = cpool.tile([128, BS], bf16)
            wb = cpool.tile([128, 32, 128], bf16)
            nc.sync.dma_start(out=csig, in_=Csig_d[:, :])
            nc.sync.dma_start(out=esig, in_=Esig_d[:, :])
            nc.sync.dma_start(out=wb, in_=WBt_d[:, :, :])

            # Input: one big [s, b, j] tile, DMA'd in quarters for pipelining.
            xin = xpool.tile([BS, BC, KI], bf16)
            for q in range(NQ):
                nc.sync.dma_start(
                    out=xin[:, q * BQ:(q + 1) * BQ, :],
                    in_=xT[:, q * BQ:(q + 1) * BQ, :],
                )

            xf = xfpool.tile([128, BC, KI], bf16)  # [(Q,c), b, j]
            z = zpool.tile([128, BC, KI], bf16)    # [(Q,j), b, c]
            yz = yzpool.tile([128, BC, 32], bf16)  # [(Q,i), b, c]
            yw = ywpool.tile([128, BC, 32], bf16)  # [(Q,c), b, i]

            # ---- stage A: DFT with (b, j)-mixed moving free dims; PSUM has
            # j innermost, so the cast drain to xf is contiguous on BOTH
            # sides (the old layout's strided drains cost ~1.7us each).
            for t in range(BC // ABT):
                ps = psApool.tile([128, ABT, KI], f32, tag="psA",
                                  name=f"psa{t}")
                for u in range(ABT // AB):
                    b0 = t * ABT + u * AB
                    nc.tensor.matmul(
                        ps[:, u * AB:(u + 1) * AB, :],
                        csig[:, :],
                        xin[:, b0:b0 + AB, :],
                        start=True, stop=True,
                    )
                drain(xf[:, t * ABT:(t + 1) * ABT, :], ps,
                      "v" if t % 4 == 3 else "s")

            # ---- T1: packed bf16 stream transpose, quarter slabs.
            for q in range(NQ):
                nc.vector.transpose(
                    out=z[:, q * BQ:(q + 1) * BQ, :],
                    in_=xf[:, q * BQ:(q + 1) * BQ, :],
                )

            # ---- stage B: per-slot block-diagonal matmul, 4 slots per
            # 2-bank PSUM tile; Scalar/GpSimd drain (b,c)-ordered into yz.
            for h in range(NH):
                for g in range(32 // CQ):
                    ps = psBpool.tile([128, CQ, BH], f32, tag="psB",
                                      name=f"psb{h}_{g}")
                    for k in range(CQ):
                        c = g * CQ + k
                        nc.tensor.matmul(
                            ps[:, k, :],
                            wb[:, c, :],
                            z[:, h * BH:(h + 1) * BH, c],
                            start=True, stop=True,
                        )
                    drain(
                        yz[:, h * BH:(h + 1) * BH, g * CQ:(g + 1) * CQ],
                        ps.rearrange("p c b -> p b c"),
                        "s",
                    )

            # ---- T2 (quarter slabs) + stage C + output DMA.
            def t2(q):
                nc.vector.transpose(
                    out=yw[:, q * BQ:(q + 1) * BQ, :],
                    in_=yz[:, q * BQ:(q + 1) * BQ, :],
                )

            def stage_c(q):
                # iDFT; esig stationary shared; 4 output blocks per matmul.
                oq = oqpool.tile([BS, KO, BQ], bf16, tag="oq", name=f"oq{q}")
                for i in range(0, KO, CQ):
                    ps = psCpool.tile([128, BQ, CQ], f32, tag="psC",
                                      name=f"psc{q}_{i}")
                    nc.tensor.matmul(
                        ps, esig[:, :],
                        yw[:, q * BQ:(q + 1) * BQ, i:i + CQ],
                        start=True, stop=True,
                    )
                    drain(
                        oq[:, i:i + CQ, :],
                        ps.rearrange("p b i -> p i b"),
                        "v" if (i // CQ) % 4 == 3 else "s",
                    )
                nc.sync.dma_start(out=outT[:, q, :, :], in_=oq)

            # Interleave so the PE keeps working and each quarter's output
            # DMA starts as soon as its drains land.
            for q in range(NQ):
                t2(q)
                stage_c(q)

    nc.compile()
    _NC_CACHE["fft"] = nc
    return nc


def _prep_fft(x, W, D, bias):
    import ml_dtypes
    bf16 = ml_dtypes.bfloat16
    Csig, Esig = _pack_const()
    WBt = _pack_wb(W)
    xd = (x * D[None, :]).astype(bf16)  # fold Bernoulli diagonal host-side
    Csig16 = Csig.astype(bf16)
    Esig16 = Esig.astype(bf16)
    WBt16 = WBt.astype(bf16)
    in_maps = []
    for c in range(NCORES):
        xs = xd[c * BC:(c + 1) * BC, :]
        # [b, j, s] -> [s, b, j]
        xTc = np.ascontiguousarray(xs.reshape(BC, KI, BS).transpose(2, 0, 1))
        im = {"xT": xTc, "Csig": Csig16, "WBt": WBt16, "Esig": Esig16}
        if LDWOPT:
            im["ldwopt_tag"] = np.zeros((1, 1), dtype=np.float32)
        in_maps.append(im)
    return in_maps


# ------------------------------------------------------------------- driver
def _run(inputs, trace=False):
    x = np.asarray(inputs["x"], dtype=np.float32)
    W = np.asarray(inputs["W"], dtype=np.float32)
    D = np.asarray(inputs["D_bernoulli"], dtype=np.float32)
    bias = np.asarray(inputs["bias"], dtype=np.float32)

    nc = _build_fft()
    in_maps = _prep_fft(x, W, D, bias)

    res = run_bass_kernel_spmd(nc, in_maps, list(range(NCORES)), trace=trace)
    out = np.empty((BATCH, D_OUT), dtype=np.float32)
    for c in range(NCORES):
        oT = np.asarray(res.results[c]["outT"]).astype(np.float32)  # [s,q,i,bq]
        out[c * BC:(c + 1) * BC, :] = (
            oT.transpose(1, 3, 2, 0).reshape(BC, D_OUT)
        )
    out += bias[None, :]
    return out, res


def kernel(**inputs) -> np.ndarray:
    out, _ = _run(inputs, trace=False)
    return out


# revision 16
# speedup vs baseline: 1.7520x; 1.1581x over previous
"""Block-circulant matmul kernel for Trainium2 (8 NeuronCores, data-parallel).

Computes out = (x * D) @ M + bias where M is the 4096x4096 block-circulant
matrix built from W[32, 32, 128] (block (i,j) is C_ij[s,t] = W[i,j,(s-t)%128]).

Sharding: batch (4096) split 8 ways -> 512 rows per core; weights replicated.

Device algorithm (per core): 3-stage frequency-domain factorization.
  A: DFT-as-matmul with (b16, j32)-mixed moving free dims, so PSUM holds
     XF[m, b, j] with j innermost.
  T1: DVE stream-transpose DIRECTLY FROM PSUM, fp32 -> fp32 (the HW
     transpose cannot cast: s4d4_tr_same_src_dst_type) -> z fp32.
     No stage-A drain instructions at all; stage B reads z (and its wb
     weights) bitcast to float32r, which streams at bf16 speed for
     moving free dims >= 256.
  B: per-frequency-slot block-diagonal matmul; 4 slots share one 2-bank
     PSUM tile so the drain copy has 4-wide contiguous runs; drains on
     Scalar/GpSimd (DVE stays free for transposes).
  T2: packed DVE stream-transpose yz -> yw in quarter-slabs.
  C: iDFT-as-matmul; pure-cast drains (bias is applied host-side, it is a
     linear post-op) on Scalar/GpSimd; quarter output tiles DMA out with
     8KB/partition contiguous runs.

The sigma frequency packing puts the 4 real components of a frequency
pair-slot c at spectrum positions {c, 32+c, 64+c, 96+c} so the quadrant-local
DVE transpose lands rows exactly where the next stage's matmul needs them.

Perf notes:
 - Whole dataflow bf16 (PSUM accumulation fp32).
 - The old kernel spent ~200us of Scalar/DVE time on strided-destination
   PSUM->SBUF copies; T1-from-PSUM plus c-quad PSUM grouping removes them.
 - DVE is the bottleneck engine (~40us of transposes); everything else is
   kept off DVE and ordered to hide under it.
"""

import os
import numpy as np

import concourse.bass as bass
import concourse.mybir as mybir
from concourse import bacc
from concourse.tile import TileContext
from concourse.bass_utils import run_bass_kernel_spmd
import concourse.bass_utils as _bu

# Walrus flag rewrites for this kernel's own compiles:
#  - LDWOPT: let walrus overlap LDWEIGHTS with in-flight matmuls.
#  - SKIP_BIRVER: drop the birverifier pass (the dtype-casting DVE stream
#    transpose trips its tag checks; correctness is checked end-to-end).
LDWOPT = os.environ.get("BC_LDWOPT", "0") == "1"
SKIP_BIRVER = os.environ.get("BC_SKIP_BIRVER", "1") == "1"
if not getattr(_bu, "_bc_ldwopt_patched", False):
    _bu._bc_ldwopt_patched = True
    _orig_bvo = _bu.bir_verify_and_optimise

    def _bvo_ldwopt(*a, **k):
        orig_rc = _bu.run_command

        def rc(argv, **kw):
            def rw(s):
                if LDWOPT:
                    s = s.replace("--enable-ldw-opt=false",
                                  "--enable-ldw-opt=true")
                if SKIP_BIRVER and s.startswith("birverifier,"):
                    s = s[len("birverifier,"):]
                return s

            return orig_rc([rw(s) for s in argv], **kw)

        _bu.run_command = rc
        try:
            return _orig_bvo(*a, **k)
        finally:
            _bu.run_command = orig_rc

    _bu.bir_verify_and_optimise = _bvo_ldwopt

# Problem constants (hardcoded per harness contract).
BATCH = 4096
D_IN = 4096
D_OUT = 4096
BS = 128          # circulant block size
KI = 32           # input blocks
KO = 32           # output blocks
NCORES = 8
BC = BATCH // NCORES      # 512 batch rows per core

NQ = 4                    # output quarters (b-blocks of 128)
BQ = BC // NQ             # 128
NH = 2                    # stage-B halves (b-blocks of 256)
BH = BC // NH             # 256
AB = 16                   # stage-A b-block per matmul (AB*KI = 512 free)
ABT = 32                  # stage-A b-block per T1 transpose (2 matmuls)
CQ = 4                    # stage-B slots per PSUM tile

_NC_CACHE = {}
_PACK_CACHE = {}


# ---------------------------------------------------------------- sigma pack
def _sigma_components():
    """slot c, quadrant Q -> ("re"|"im", f). Pairs (2c+1, 2c+2) for c<31,
    slot 31 holds (63 complex, 0 real, 64 real)."""
    comp = {}
    for c in range(32):
        fa = 2 * c + 1 if c < 31 else 63
        comp[(0, c)] = ("re", fa)
        comp[(1, c)] = ("im", fa)
        if c < 31:
            comp[(2, c)] = ("re", 2 * c + 2)
            comp[(3, c)] = ("im", 2 * c + 2)
        else:
            comp[(2, c)] = ("re", 0)
            comp[(3, c)] = ("re", 64)
    return comp


def _pack_const():
    """Input-independent factor matrices Csig [s, m] and Esig [m, t]."""
    if "const" in _PACK_CACHE:
        return _PACK_CACHE["const"]
    comp = _sigma_components()
    s = np.arange(BS)
    Csig = np.zeros((BS, 128), dtype=np.float64)
    Esig = np.zeros((128, BS), dtype=np.float64)
    for (Q, c), (typ, f) in comp.items():
        m = 32 * Q + c
        ang = 2 * np.pi * f * s / BS
        a = (1.0 if f in (0, 64) else 2.0) / BS
        if typ == "re":
            Csig[:, m] = np.cos(ang)
            Esig[m, :] = a * np.cos(ang)
        else:
            Csig[:, m] = -np.sin(ang)
            Esig[m, :] = -a * np.sin(ang)
    out = (Csig.astype(np.float32), np.ascontiguousarray(Esig.astype(np.float32)))
    _PACK_CACHE["const"] = out
    return out


def _pack_wb(W):
    """Frequency-domain block-diagonal weights WBt [row=(Qr,j), slot, col=(Qc,i)]."""
    comp = _sigma_components()
    Wf = np.fft.fft(W.astype(np.float64), axis=-1)
    Wfr, Wfi = Wf.real, Wf.imag
    WB = np.zeros((32, 128, 128), dtype=np.float64)
    for c in range(32):
        for (qre, qim) in ((0, 1), (2, 3)):
            typ_im = comp[(qim, c)][0]
            f = comp[(qre, c)][1]
            if typ_im == "im":
                wr = Wfr[:, :, f].T  # [j, i]
                wi = Wfi[:, :, f].T
                WB[c, qre*32:(qre+1)*32, qre*32:(qre+1)*32] = wr
                WB[c, qim*32:(qim+1)*32, qre*32:(qre+1)*32] = wi
                WB[c, qre*32:(qre+1)*32, qim*32:(qim+1)*32] = -wi
                WB[c, qim*32:(qim+1)*32, qim*32:(qim+1)*32] = wr
            else:
                f2 = comp[(qim, c)][1]
                WB[c, qre*32:(qre+1)*32, qre*32:(qre+1)*32] = Wfr[:, :, f].T
                WB[c, qim*32:(qim+1)*32, qim*32:(qim+1)*32] = Wfr[:, :, f2].T
    return np.ascontiguousarray(
        WB.transpose(1, 0, 2).astype(np.float32)  # [row, slot, col]
    )


# ---------------------------------------------------------------- build
def _build_fft():
    if "fft" in _NC_CACHE:
        return _NC_CACHE["fft"]
    f32 = mybir.dt.float32
    bf16 = mybir.dt.bfloat16

    nc = bacc.Bacc(None, target_bir_lowering=False, debug=False)

    # D_bernoulli is folded into x host-side; bias is applied host-side.
    f32r = mybir.dt.float32r
    xT = nc.dram_tensor("xT", [BS, BC, KI], bf16, kind="ExternalInput")
    Csig_d = nc.dram_tensor("Csig", [BS, 128], bf16, kind="ExternalInput")
    WBt_d = nc.dram_tensor("WBt", [128, 32, 128], f32, kind="ExternalInput")
    Esig_d = nc.dram_tensor("Esig", [128, BS], bf16, kind="ExternalInput")
    # [s, q, i, bq]: each quarter DMA writes 8KB contiguous per partition.
    outT = nc.dram_tensor("outT", [BS, NQ, KO, BQ], bf16, kind="ExternalOutput")
    if LDWOPT:
        nc.dram_tensor("ldwopt_tag", [1, 1], f32, kind="ExternalInput")

    def drain(out, in_, eng):
        # GpSimd has no PSUM access on TRN2, so PSUM drains are split
        # between Scalar ("s") and DVE ("v") only.
        if eng == "s":
            nc.scalar.activation(
                out=out, in_=in_, func=mybir.ActivationFunctionType.Copy
            )
        else:
            nc.vector.tensor_copy(out=out, in_=in_)

    with TileContext(nc) as tc:
        with tc.tile_pool(name="consts", bufs=1) as cpool, \
             tc.tile_pool(name="xin", bufs=1) as xpool, \
             tc.tile_pool(name="zb", bufs=1) as zpool, \
             tc.tile_pool(name="yzb", bufs=1) as yzpool, \
             tc.tile_pool(name="ywb", bufs=1) as ywpool, \
             tc.tile_pool(name="oq", bufs=2) as oqpool, \
             tc.tile_pool(name="psA", bufs=2, space="PSUM") as psApool, \
             tc.tile_pool(name="psB", bufs=1, space="PSUM") as psBpool, \
             tc.tile_pool(name="psC", bufs=2, space="PSUM") as psCpool:

            csig = cpool.tile([BS, 128], bf16)
            esig = cpool.tile([128, BS], bf16)
            wb = cpool.tile([128, 32, 128], f32)
            xin = xpool.tile([BS, BC, KI], bf16)

            # Order the DMAs so the first stage-A matmul unblocks ASAP:
            # csig + first input chunk first, bulky weights after.
            NCH = 8
            BCH = BC // NCH
            nc.sync.dma_start(out=csig, in_=Csig_d[:, :])
            nc.sync.dma_start(out=xin[:, 0:BCH, :], in_=xT[:, 0:BCH, :])
            nc.sync.dma_start(out=xin[:, BCH:2 * BCH, :],
                              in_=xT[:, BCH:2 * BCH, :])
            nc.sync.dma_start(out=esig, in_=Esig_d[:, :])
            nc.sync.dma_start(out=wb, in_=WBt_d[:, :, :])
            for ch in range(2, NCH):
                nc.sync.dma_start(
                    out=xin[:, ch * BCH:(ch + 1) * BCH, :],
                    in_=xT[:, ch * BCH:(ch + 1) * BCH, :],
                )

            z = zpool.tile([128, BC, KI], f32)     # [(Q,j), b, c] fp32
            yz = yzpool.tile([128, BC, 32], bf16)  # [(Q,i), b, c]
            yw = ywpool.tile([128, BC, 32], bf16)  # [(Q,c), b, i]

            # ---- stage A + T1: DFT, then stream-transpose straight out of
            # PSUM into z (fp32 -> fp32, dtype-preserving).
            for t in range(BC // ABT):
                ps = psApool.tile([128, ABT, KI], f32, tag="psA",
                                  name=f"psa{t}")
                for u in range(ABT // AB):
                    b0 = t * ABT + u * AB
                    nc.tensor.matmul(
                        ps[:, u * AB:(u + 1) * AB, :],
                        csig[:, :],
                        xin[:, b0:b0 + AB, :],
                        start=True, stop=True,
                    )
                nc.vector.transpose(
                    out=z[:, t * ABT:(t + 1) * ABT, :], in_=ps
                )

            # ---- stage B: per-slot block-diagonal matmul, 4 slots per
            # 2-bank PSUM tile; Scalar/GpSimd drain (b,c)-ordered into yz.
            for h in range(NH):
                for g in range(32 // CQ):
                    ps = psBpool.tile([128, CQ, BH], f32, tag="psB",
                                      name=f"psb{h}_{g}")
                    for k in range(CQ):
                        c = g * CQ + k
                        nc.tensor.matmul(
                            ps[:, k, :],
                            wb[:, c, :].bitcast(f32r),
                            z[:, h * BH:(h + 1) * BH, c].bitcast(f32r),
                            start=True, stop=True,
                        )
                    drain(
                        yz[:, h * BH:(h + 1) * BH, g * CQ:(g + 1) * CQ],
                        ps.rearrange("p c b -> p b c"),
                        "s",
                    )

            # ---- T2 (quarter slabs) + stage C + output DMA.
            def t2(q):
                nc.vector.transpose(
                    out=yw[:, q * BQ:(q + 1) * BQ, :],
                    in_=yz[:, q * BQ:(q + 1) * BQ, :],
                )

            def stage_c(q):
                # iDFT; esig stationary shared; 4 output blocks per matmul.
                oq = oqpool.tile([BS, KO, BQ], bf16, tag="oq", name=f"oq{q}")
                for i in range(0, KO, CQ):
                    ps = psCpool.tile([128, BQ, CQ], f32, tag="psC",
                                      name=f"psc{q}_{i}")
                    nc.tensor.matmul(
                        ps, esig[:, :],
                        yw[:, q * BQ:(q + 1) * BQ, i:i + CQ],
                        start=True, stop=True,
                    )
                    drain(
                        oq[:, i:i + CQ, :],
                        ps.rearrange("p b i -> p i b"),
                        "v" if (i // CQ) % 4 == 3 else "s",
                    )
                nc.sync.dma_start(out=outT[:, q, :, :], in_=oq)

            # Interleave so the PE keeps working and each quarter's output
            # DMA starts as soon as its drains land.
            for q in range(NQ):
                t2(q)
                stage_c(q)

    nc.compile()
    _NC_CACHE["fft"] = nc
    return nc


def _prep_fft(x, W, D, bias):
    import ml_dtypes
    bf16 = ml_dtypes.bfloat16
    Csig, Esig = _pack_const()
    WBt = _pack_wb(W)
    xd = (x * D[None, :]).astype(bf16)  # fold Bernoulli diagonal host-side
    Csig16 = Csig.astype(bf16)
    Esig16 = Esig.astype(bf16)
    WBt32 = WBt.astype(np.float32)  # f32r (fp32 bits) on device
    in_maps = []
    for c in range(NCORES):
        xs = xd[c * BC:(c + 1) * BC, :]
        # [b, j, s] -> [s, b, j]
        xTc = np.ascontiguousarray(xs.reshape(BC, KI, BS).transpose(2, 0, 1))
        im = {"xT": xTc, "Csig": Csig16, "WBt": WBt32, "Esig": Esig16}
        if LDWOPT:
            im["ldwopt_tag"] = np.zeros((1, 1), dtype=np.float32)
        in_maps.append(im)
    return in_maps


# ------------------------------------------------------------------- driver
def _run(inputs, trace=False):
    x = np.asarray(inputs["x"], dtype=np.float32)
    W = np.asarray(inputs["W"], dtype=np.float32)
    D = np.asarray(inputs["D_bernoulli"], dtype=np.float32)
    bias = np.asarray(inputs["bias"], dtype=np.float32)

    nc = _build_fft()
    in_maps = _prep_fft(x, W, D, bias)

    res = run_bass_kernel_spmd(nc, in_maps, list(range(NCORES)), trace=trace)
    out = np.empty((BATCH, D_OUT), dtype=np.float32)
    for c in range(NCORES):
        oT = np.asarray(res.results[c]["outT"]).astype(np.float32)  # [s,q,i,bq]
        out[c * BC:(c + 1) * BC, :] = (
            oT.transpose(1, 3, 2, 0).reshape(BC, D_OUT)
        )
    out += bias[None, :]
    return out, res


def kernel(**inputs) -> np.ndarray:
    out, _ = _run(inputs, trace=False)
    return out
